# revision 26
# baseline (speedup 1.0000x reference)
"""AdditiveAttention on 8 TRN2 NeuronCores.

Math: out = softmax_k(mask(sum_h w_v[h] * tanh(qp[b,q,h] + kp[b,k,h]))) @ values
with qp = queries @ W_q^T, kp = keys @ W_k^T, mask from valid_lens (B,).

tanh(u) ~= sum_{r in RS} b_r sin(r*w0*u), RS=[1,2,3,4,6], fit per batch.
sin(r*w0*(q+k)) factorizes by angle addition, so scores come from 4R matmuls
with contraction over h instead of a (B,Q,K,H) tensor.

Harmonics: ACT Sin gives s1/c1 of qp,kp; DVE/Pool recurrences give the rest:
  sq1=s1*s1; m3=3-4sq1; m1=1-4sq1; c2=1-2sq1       (sin3=s1*m3, cos3=c1*m1)
  s2=s1*c1 (=sin2/2); s4=s2*c2 (=sin4/4); s6=s3*c3 (=sin6/2)
  c4=1-8*s2^2; c6=1-2*s3^2                         (squares on ACT)
Stored sin_r is scaled by 2^-A[r]; the q-side stationary scale columns carry
wv[h]*b_r*2^A[r], which also compensates the k-side moving sin scaling.

Softmax: exp(score - 4.16) on ACT straight from PSUM (no row-max pass; the
harmonic score bound keeps exp in fp16 range), accum_out gives the row sum,
masked key columns get -60000 via a rank-1 matmul so exp is exactly 0.

Sharding: core c handles batch c//2, query rows (c%2)*256..+256.
The harmonic chain runs on DVE (fp16 2x tensor_tensor / 4x tensor_scalar);
squares go to ACT's Square in its idle gaps; one slack tensor_scalar (m3)
goes to GpSimd. Eight warm matmuls after the projections keep the PE busy
through the trig wait so the DVFS ramp to 2.4GHz completes before the
score matmuls. The 257th column of V is all-ones, making av[:,256] the
softmax denominator at no extra cost.
"""

import math
from contextlib import ExitStack

import numpy as np

import concourse.bass as bass
import concourse.mybir as mybir
import concourse.tile as tile
from concourse import bacc
from concourse.bass_utils import run_bass_kernel_spmd

B, Q, K, D, H, V = 4, 512, 512, 256, 256, 256
NCORES = 8
NQ = (B * Q) // NCORES          # 256 query rows per core
RS = [1, 2, 3, 4, 6]
NR = len(RS)
A_EXP = {1: 0, 2: 1, 3: 0, 4: 2, 6: 1}
NEGM = -60000.0                 # mask add (exp -> exactly 0)
EBIAS = -4.16                   # exp bias: p = e^(s-4.16) stays in fp16 range
FP32 = mybir.dt.float32
FP16 = mybir.dt.float16
AX = mybir.AxisListType
ALU = mybir.AluOpType
ACTF = mybir.ActivationFunctionType


def fit_series(qp_b, kp_bv, wsig=1.5):
    """Least-squares harmonic fit for one batch. qp_b/kp_bv: [h,*] valid."""
    umax = max((qp_b.max(1) + kp_bv.max(1)).max(),
               -(qp_b.min(1) + kp_bv.min(1)).min())
    xmax = max(np.abs(qp_b).max(), np.abs(kp_bv).max())
    P = max(2.0 * (umax + 0.15), 4.0 * xmax + 0.08)
    w0 = 2.0 * np.pi / P
    u = np.linspace(-(umax + 0.05), umax + 0.05, 4001)
    A = np.stack([np.sin(r * w0 * u) for r in RS], 1)
    wgt = np.exp(-(u ** 2) / (2 * wsig ** 2)) + 1e-3
    sw = np.sqrt(wgt)[:, None]
    bco, *_ = np.linalg.lstsq(A * sw, np.tanh(u) * sw[:, 0], rcond=None)
    return float(w0), bco.astype(np.float64)


def pack_layout(KP):
    NK = KP // 128
    names = ([("wq0", H), ("wq1", H), ("qT0", NQ), ("qT1", NQ),
              ("wk0", H), ("wk1", H), ("kT0", KP), ("kT1", KP)]
             + [(f"v{i}", V + 1) for i in range(NK)] + [("ident", 128)])
    off, x = {}, 0
    for nm, w in names:
        off[nm] = x
        x += w
    return off, x


class TileCtx:
    def __init__(self, nc):
        self.nc = nc

    def __enter__(self):
        self.ctx = ExitStack()
        self.tc = self.ctx.enter_context(tile.TileContext(self.nc))
        return self.tc, self.ctx

    def __exit__(self, *exc):
        return self.ctx.__exit__(*exc)


def build_nc(w0s, bcos, KP):
    NK = KP // 128
    QW = 2 * NQ                    # q-region width (both h-chunks)
    CW = QW + 2 * KP               # harmonic tile width: [q hc0|q hc1|k hc0|k hc1]
    OFF, PX = pack_layout(KP)
    NCOL = 2 * NR + 1              # scale cols (hc-major) + w0

    nc = bacc.Bacc()
    pack = nc.declare_dram_parameter("pack", [128, PX], FP16, isOutput=False)
    mo = nc.declare_dram_parameter("mo", [1, KP + 128], FP16, isOutput=False)
    cols = nc.declare_dram_parameter("cols", [128, NCOL], FP32, isOutput=False)
    out_d = nc.declare_dram_parameter("out", [NQ, V], FP32, isOutput=True)

    with TileCtx(nc) as (tc, ctx):
        inp = ctx.enter_context(tc.tile_pool(name="inp", bufs=1))
        harm = ctx.enter_context(tc.tile_pool(name="harm", bufs=1))
        qbp = ctx.enter_context(tc.tile_pool(name="qb", bufs=1))
        sm = ctx.enter_context(tc.tile_pool(name="sm", bufs=1))
        ps_q = ctx.enter_context(tc.tile_pool(name="psQ", bufs=1, space="PSUM"))
        ps_k = ctx.enter_context(tc.tile_pool(name="psK", bufs=1, space="PSUM"))
        ps_sc = ctx.enter_context(tc.tile_pool(name="psS", bufs=1, space="PSUM"))
        ps_x = ctx.enter_context(tc.tile_pool(name="psX", bufs=1, space="PSUM"))

        # ---- input DMAs: projection weights/data first (critical path),
        # mask next (early mask matmuls), kT split so kp can start sooner ----
        big = inp.tile([128, PX], FP16, tag="big", name="big")
        cut1, cut2 = OFF["wk0"], OFF["v0"]
        cutk = OFF["kT1"]
        nc.sync.dma_start(out=big[:, :cut1], in_=pack[:, :cut1])
        nc.sync.dma_start(out=big[:, cut1:cutk], in_=pack[:, cut1:cutk])
        nc.sync.dma_start(out=big[:, cutk:cut2], in_=pack[:, cutk:cut2])
        cols_sb = inp.tile([128, NCOL], FP32, tag="cols", name="cols_sb")
        nc.sync.dma_start(out=cols_sb, in_=cols[:, :])
        mo_sb = inp.tile([1, KP + 128], FP16, tag="mo", name="mo_sb")
        nc.sync.dma_start(out=mo_sb, in_=mo[:, :])
        nc.sync.dma_start(out=big[:, cut2:], in_=pack[:, cut2:])

        wq_sb = [big[:, OFF[f"wq{i}"]: OFF[f"wq{i}"] + H] for i in range(2)]
        qT_sb = [big[:, OFF[f"qT{i}"]: OFF[f"qT{i}"] + NQ] for i in range(2)]
        wk_sb = [big[:, OFF[f"wk{i}"]: OFF[f"wk{i}"] + H] for i in range(2)]
        kT_sb = [big[:, OFF[f"kT{i}"]: OFF[f"kT{i}"] + KP] for i in range(2)]
        v_sb = [big[:, OFF[f"v{i}"]: OFF[f"v{i}"] + V + 1] for i in range(NK)]
        ident = big[:, OFF["ident"]: OFF["ident"] + 128]
        mrow = mo_sb[:, :KP]
        ones_r = mo_sb[:, KP: KP + 128]

        w0col = cols_sb[:, 2 * NR: 2 * NR + 1]
        hpi = inp.tile([128, 1], FP32, tag="hpi", name="hpi")
        nc.gpsimd.memset(hpi, math.pi / 2)
        ebias = inp.tile([128, 1], FP32, tag="eb", name="ebias")
        nc.gpsimd.memset(ebias, EBIAS)
        warm = inp.tile([1, 128], FP16, tag="warm", name="warm")
        # sin-table load while DMAs run (input: first DMA chunk, lands earliest)
        nc.scalar.activation(warm, big[0:1, 0:128], ACTF.Sin, scale=0.001)

        # ---- PE warm-up: dummy matmuls to finish the DVFS ramp early ----
        scratch = ps_k.tile([128, KP], FP32, tag="kp0", name="scratch")
        for i in range(NDUMMY):
            nc.tensor.matmul(scratch, ones_r, mrow, start=True, stop=True)

        # ---- mask rows into score PSUMs (opens accumulation groups), then
        # warm matmuls so the PE DVFS ramp finishes before the score phase ----
        sc_ps = [ps_sc.tile([128, KP], FP32, tag=f"sc{qt}", name=f"sc{qt}")
                 for qt in range(2)]
        for qt in range(2):
            nc.tensor.matmul(sc_ps[qt], ones_r, mrow, start=True, stop=False)
        scratch = ps_x.tile([128, KP], FP32, tag="xx", name="scratch")
        for _ in range(8):
            nc.tensor.matmul(scratch, ones_r, mrow, start=True, stop=True)

        # ---- projections ----
        qp_ps = ps_q.tile([128, QW], FP32, tag="qp", name="qp")
        for hc in range(2):
            for dc in range(2):
                nc.tensor.matmul(qp_ps[:, hc * NQ:(hc + 1) * NQ],
                                 wq_sb[dc][:, 128 * hc: 128 * (hc + 1)],
                                 qT_sb[dc], start=(dc == 0), stop=(dc == 1))
        # kp as one 2-bank tile [128, 2, 512]: each h-chunk's 384 cols sit in
        # its own bank; one strided ACT read covers both chunks per trig op
        kp_ps = ps_k.tile([128, 2, 512], FP32, tag="kp", name="kp")
        for hc in range(2):
            for dc in range(2):
                nc.tensor.matmul(kp_ps[:, hc, :KP],
                                 wk_sb[dc][:, 128 * hc: 128 * (hc + 1)],
                                 kT_sb[dc], start=(dc == 0), stop=(dc == 1))

        # ---- base harmonics: s1/c1 via ACT Sin (args within table range) ----
        sc = {r: harm.tile([128, 2, CW], FP16, tag=f"sc{r}", name=f"sc{r}")
              for r in RS}
        s = {r: sc[r][:, 0] for r in RS}
        c = {r: sc[r][:, 1] for r in RS}
        ksl = [slice(QW + hc * KP, QW + (hc + 1) * KP) for hc in range(2)]
        kall = slice(QW, QW + 2 * KP)
        kp_in = kp_ps[:, :, :KP]
        nc.scalar.activation(s[1][:, :QW], qp_ps, ACTF.Sin, scale=w0col)
        nc.scalar.activation(c[1][:, :QW], qp_ps, ACTF.Sin, scale=w0col, bias=hpi)
        nc.scalar.activation(s[1][:, kall], kp_in, ACTF.Sin, scale=w0col)
        nc.scalar.activation(c[1][:, kall], kp_in, ACTF.Sin, scale=w0col,
                             bias=hpi)

        sq = {m: harm.tile([128, CW], FP16, tag=f"sq{m}", name=f"sq{m}")
              for m in (1, 2, 3)}
        m1 = harm.tile([128, CW], FP16, tag="m1", name="m1")
        m3 = harm.tile([128, CW], FP16, tag="m3", name="m3")

        tt = nc.vector.tensor_mul

        def tsp(out, in_, mul, add):
            nc.vector.tensor_scalar(out, in_, mul, add, ALU.mult, ALU.add)

        # ---- q-side b-scaled stationaries: one 4x tensor_scalar per (r,hc) ----
        SCb = {r: qbp.tile([128, 2, QW], FP16, tag=f"SCb{r}", name=f"SCb{r}")
               for r in RS}

        def scale_r(r):
            j = RS.index(r)
            for hc in range(2):
                qsl = slice(hc * NQ, (hc + 1) * NQ)
                nc.vector.tensor_scalar(SCb[r][:, :, qsl], sc[r][:, :, qsl],
                                        cols_sb[:, hc * NR + j: hc * NR + j + 1],
                                        None, ALU.mult)

        # ---- warm matmuls so the PE DVFS ramp finishes before the scores ----
        scratch = ps_x.tile([128, KP], FP32, tag="xx", name="scratch")
        for _ in range(8):
            nc.tensor.matmul(scratch, ones_r, mrow, start=True, stop=True)

        # ---- transposed score matmuls: psT[kc][k, q] accumulates
        # raw-k-trig (stationary) x scaled-q-trig (moving); masking comes
        # free from the zeroed ones/V rows at padded keys ----
        scT_ps = [ps_sc.tile([128, NQ], FP32, tag=f"scT{kc}", name=f"scT{kc}")
                  for kc in range(NK)]

        def mm_r(r):
            first, last = r == RS[0], r == RS[-1]
            for hc in range(2):
                qs = slice(hc * NQ, (hc + 1) * NQ)
                for kc in range(NK):
                    kst = slice(QW + hc * KP + 128 * kc,
                                QW + hc * KP + 128 * (kc + 1))
                    nc.tensor.matmul(scT_ps[kc], c[r][:, kst], SCb[r][:, 0, qs],
                                     start=(first and hc == 0), stop=False)
                    nc.tensor.matmul(scT_ps[kc], s[r][:, kst], SCb[r][:, 1, qs],
                                     start=False, stop=(last and hc == 1))

        # DVE queue ordered by operand readiness
        tt(sq[1], s[1], s[1])
        scale_r(1)
        mm_r(1)
        tsp(c[2], sq[1], -2.0, 1.0)
        tt(s[2], s[1], c[1])
        scale_r(2)
        mm_r(2)
        nc.gpsimd.tensor_scalar(m3, sq[1], -4.0, 3.0, ALU.mult, ALU.add)
        tsp(m1, sq[1], -4.0, 1.0)
        tt(s[3], s[1], m3)
        tt(c[3], c[1], m1)
        scale_r(3)
        mm_r(3)
        # squares for c4/c6/c8 on ACT (Square is in the sin table set)
        nc.scalar.activation(sq[2], s[2], ACTF.Square)
        nc.scalar.activation(sq[3], s[3], ACTF.Square)
        tsp(c[4], sq[2], -8.0, 1.0)
        tt(s[4], s[2], c[2])
        scale_r(4)
        mm_r(4)
        tsp(c[6], sq[3], -2.0, 1.0)
        tt(s[6], s[3], c[3])
        scale_r(6)
        # exp-table swap; input dep on sq[3] pins it after the last Square
        nc.scalar.activation(warm, sq[3][0:1, 0:128], ACTF.Exp)
        mm_r(6)

        # ---- softmax + AV per q-tile ----
        # exp writes p^T [k, q] directly; AV needs no transposes.  V carries
        # a 257th column that is 1 on valid rows and 0 on padding, so av[:, V]
        # is the masked softmax denominator for free
        pT = [sm.tile([128, NQ], FP16, tag=f"pT{kc}", name=f"pT{kc}")
              for kc in range(NK)]
        for kc in range(NK):
            nc.scalar.activation(pT[kc], scT_ps[kc], ACTF.Exp, bias=ebias)
        for qt in range(2):
            av = ps_q.tile([128, V + 1], FP32, tag="qp", name=f"av{qt}")
            for kc in range(NK):
                nc.tensor.matmul(av, pT[kc][:, 128 * qt: 128 * (qt + 1)],
                                 v_sb[kc], start=(kc == 0), stop=(kc == NK - 1))
            rs = sm.tile([128, 1], FP32, tag=f"rs{qt}", name=f"rs{qt}")
            nc.vector.reciprocal(rs, av[:, V: V + 1])
            o_sb = sm.tile([128, V], FP32, tag=f"o{qt}", name=f"o{qt}")
            nc.scalar.activation(o_sb, av[:, :V], ACTF.Copy, scale=rs)
            nc.sync.dma_start(out=out_d[128 * qt: 128 * (qt + 1), :], in_=o_sb)

    nc.compile()
    return nc


def prepare(inputs):
    """Host prep: per-batch harmonic fit, per-core packed inputs."""
    queries = np.ascontiguousarray(np.asarray(inputs["queries"], np.float32))
    keys = np.ascontiguousarray(np.asarray(inputs["keys"], np.float32))
    values = np.ascontiguousarray(np.asarray(inputs["values"], np.float32))
    vls = np.asarray(inputs["valid_lens"]).astype(np.int64)
    Wq = np.asarray(inputs["W_q"], np.float32)
    Wk = np.asarray(inputs["W_k"], np.float32)
    wv = np.asarray(inputs["w_v"], np.float32)

    # device projections run on fp16-rounded inputs; match that for the fit
    q16 = queries.astype(np.float16).astype(np.float32)
    k16 = keys.astype(np.float16).astype(np.float32)
    Wq16 = Wq.astype(np.float16).astype(np.float32)
    Wk16 = Wk.astype(np.float16).astype(np.float32)
    qp = [(Wq16 @ q16[b].T).astype(np.float32) for b in range(B)]   # [h, q]
    kp = [(Wk16 @ k16[b].T).astype(np.float32) for b in range(B)]   # [h, k]
    fits = [fit_series(qp[b], kp[b][:, : vls[b]]) for b in range(B)]
    w0s = [f[0] for f in fits]
    bcos = [f[1] for f in fits]
    KP = 128 * max(1, int(math.ceil(vls.max() / 128.0)))

    OFF, PX = pack_layout(KP)
    NK = KP // 128
    NCOL = 2 * NR + 1
    in_maps = []
    for core in range(NCORES):
        b, qlo = core // 2, (core % 2) * NQ
        w0, bco = w0s[b], bcos[b]
        n = int(vls[b])
        colm = np.zeros((128, NCOL), np.float32)
        for hc in range(2):
            wvh = wv[128 * hc: 128 * (hc + 1)]
            for j, r in enumerate(RS):
                colm[:, hc * NR + j] = wvh * bco[j] * (2.0 ** A_EXP[r])
        colm[:, 2 * NR] = w0

        pk = np.zeros((128, PX), np.float16)
        qTm = queries[b, qlo: qlo + NQ].T.astype(np.float16)        # (D, NQ)
        kTm = np.zeros((D, KP), np.float16)
        kTm[:, :n] = keys[b, :n].T.astype(np.float16)
        for i in range(2):
            pk[:, OFF[f"qT{i}"]: OFF[f"qT{i}"] + NQ] = qTm[128 * i: 128 * (i + 1)]
            pk[:, OFF[f"kT{i}"]: OFF[f"kT{i}"] + KP] = kTm[128 * i: 128 * (i + 1)]
            pk[:, OFF[f"wq{i}"]: OFF[f"wq{i}"] + H] = Wq.T[128 * i: 128 * (i + 1)].astype(np.float16)
            pk[:, OFF[f"wk{i}"]: OFF[f"wk{i}"] + H] = Wk.T[128 * i: 128 * (i + 1)].astype(np.float16)
        vm = np.zeros((KP, V + 1), np.float16)
        vm[:n, :V] = values[b, :n].astype(np.float16)
        vm[:n, V] = 1.0
        for i in range(NK):
            pk[:, OFF[f"v{i}"]: OFF[f"v{i}"] + V + 1] = vm[128 * i: 128 * (i + 1)]
        pk[:, OFF["ident"]: OFF["ident"] + 128] = np.eye(128, dtype=np.float16)
        mov = np.zeros((1, KP + 128), np.float16)
        mov[0, :KP] = np.where(np.arange(KP) < n, 0.0, NEGM).astype(np.float16)
        mov[0, KP:] = 1.0
        in_maps.append({"pack": pk, "mo": mov, "cols": colm})
    return w0s, bcos, KP, in_maps


def kernel(**inputs):
    w0s, bcos, KP, in_maps = prepare(inputs)
    nc = build_nc(w0s, bcos, KP)
    res = run_bass_kernel_spmd(nc, in_maps, core_ids=list(range(NCORES)))
    out = np.zeros((B, Q, V), np.float32)
    for core in range(NCORES):
        b, qlo = core // 2, (core % 2) * NQ
        out[b, qlo: qlo + NQ] = res.results[core]["out"]
    return out
